# revision 12
# baseline (speedup 1.0000x reference)
"""Trainium2 Bass kernel for nn_GCEncoderLayer_78400333021790.

GC encoder layer: per-node MHA over T=12 steps + FFN (both with residual+LN),
then a 3-support graph convolution over the 325-node sensor graph.

Strategy (data-parallel over batch B=32 -> 4 batches per core, 8 cores):
  - token order per core: (b, t, n); activations kept feature-major
    X^T = (d=128 partitions, tokens free) so every projection is a natural
    PE matmul.
  - MHA algebra folded on CPU:  S^T = (X Wqk^T) X^T with Wqk = Wq Wk^T/sqrt(128)
    (bq=bk=0 per spec), Vt = X (Wv Wo) so the output projection disappears.
  - groups of 10 nodes (120 tokens) per attention step; block-diagonal mask
    realized as a rank-11 matmul pre-loaded into PSUM (exp underflows to 0).
  - softmax normalization: exp (ACT) -> partition_all_reduce (GPSIMD) ->
    reciprocal_approx_fast + multiply (DVE).
  - LayerNorm in feature-major: column sums via ones-matmuls into
    tile_position-rotated PSUM rows, reshaped to (128, x) tiles via SBUF DMA,
    per-token scale/shift broadcast back through K=1 matmuls.
  - GCN: out = Z G0 + A0 (Z G1) + A1 (Z G2) + bias with dense A built on CPU;
    the node-mix contraction runs on token-major tiles (node on partition)
    which the (b, t, n) token order provides for free.
"""

import os
import sys

for _p in ("/opt/trn_rl_repo", "/root/.axon_site/_ro/trn_rl_repo"):
    if os.path.isdir(_p) and _p not in sys.path:
        sys.path.insert(0, _p)

from contextlib import ExitStack

import ml_dtypes
import numpy as np

import concourse.bass as bass
import concourse.bass_isa as bass_isa
import concourse.tile as tile
from concourse import bacc, mybir

N = 325
T = 12
D = 128
H = 8
DFF = 512
NCORES = 8
B_TOT = 32
LN_EPS = 1e-3
SQRT_D = float(np.sqrt(128.0))

BF = mybir.dt.bfloat16
F32 = mybir.dt.float32
F32R = mybir.dt.float32r
AL = mybir.AluOpType
AF = mybir.ActivationFunctionType
bf16 = ml_dtypes.bfloat16

NODE_TILES = [(0, 128), (128, 128), (256, 69)]
GROUPS = [(i * 10, 10) for i in range(32)] + [(320, 5)]
BIG = 173.0  # sqrt(~30000); exp(-BIG^2) == 0 in fp32


def _r(x):
    return np.ascontiguousarray(x)


def _bf(x):
    return _r(np.asarray(x, np.float32).astype(bf16))


def make_consts(inp):
    """CPU-side weight folding. Returns dict of extra dram inputs (shared
    across cores)."""
    Wq = np.asarray(inp["Wq"], np.float32)
    Wk = np.asarray(inp["Wk"], np.float32)
    Wv = np.asarray(inp["Wv"], np.float32)
    Wo = np.asarray(inp["Wo"], np.float32)
    bv = np.asarray(inp["bv"], np.float32)
    bo = np.asarray(inp["bo"], np.float32)

    # wqkT[:, h*D:(h+1)*D][d, e] = Wqk_h[e, d],  Wqk_h = Wq_h Wk_h^T / sqrt(D)
    wqkT = np.empty((D, H * D), np.float32)
    wvo = np.empty((D, H * D), np.float32)
    for h in range(H):
        wqk_h = (Wq[:, h, :] @ Wk[:, h, :].T) / SQRT_D  # (D, D)
        wqkT[:, h * D:(h + 1) * D] = wqk_h.T
        wvo[:, h * D:(h + 1) * D] = Wv[:, h, :] @ Wo[h]  # (D, D)
    bvo = (np.einsum("hk,hkd->d", bv, Wo) + bo).astype(np.float32)

    # block-diag mask via rank-11 outer product: sum_p A[p,s] B[p,t]
    # = BIG^2*[node(s)==node(t)] - BIG^2
    bma = np.zeros((11, 120), np.float32)
    bmb = np.zeros((11, 120), np.float32)
    for blk in range(10):
        bma[blk, blk * 12:(blk + 1) * 12] = BIG
        bmb[blk, blk * 12:(blk + 1) * 12] = BIG
    bma[10, :] = BIG
    bmb[10, :] = -BIG

    A0 = np.zeros((N, N), np.float32)
    A1 = np.zeros((N, N), np.float32)
    np.add.at(A0, (np.asarray(inp["sup0_rows"]), np.asarray(inp["sup0_cols"])),
              np.asarray(inp["sup0_vals"], np.float32))
    np.add.at(A1, (np.asarray(inp["sup1_rows"]), np.asarray(inp["sup1_cols"])),
              np.asarray(inp["sup1_vals"], np.float32))

    G = np.asarray(inp["gc_kernel"], np.float32)  # (3D, D), rows ordered (d, m)
    G0, G1, G2 = G[0::3], G[1::3], G[2::3]  # each (D, D)

    w2 = np.asarray(inp["ffn_W2"], np.float32)  # (DFF, D)
    w2r = w2.reshape(4, 128, D).transpose(1, 0, 2)  # (128, 4, D)
    fb1r = np.asarray(inp["ffn_b1"], np.float32).reshape(4, 128).T  # (128, 4)

    consts = {
        "wqkT": _bf(wqkT),
        "wvo": _bf(wvo),
        "bma": _bf(bma),
        "bmb": _bf(bmb),
        "w1": _bf(inp["ffn_W1"]),
        "w2r": _bf(w2r),
        "g0": _bf(G0),
        "g12": _bf(np.concatenate([G1, G2], axis=1)),
        "a0t": _bf(A0.T),
        "a1t": _bf(A1.T),
        "gcb4": _bf(np.tile(np.asarray(inp["gc_bias"], np.float32), 4)[None, :]),
        "ident": _r(np.eye(128, dtype=np.float32)),
        "bvo": _r(bvo[:, None]),
        "fb1r": _r(fb1r),
        "fb2": _r(np.asarray(inp["ffn_b2"], np.float32)[:, None]),
        "lng1c": _r(np.asarray(inp["ln1_g"], np.float32)[:, None]),
        "lng2c": _r(np.asarray(inp["ln2_g"], np.float32)[:, None]),
    }
    return consts


def build_module(bs):
    """Emit the Bass/Tile program for one core handling `bs` batches."""
    TT = bs * T * N
    nc = bacc.Bacc("TRN2")

    x_d = nc.dram_tensor("x", [bs * N, T, D], F32, kind="ExternalInput")
    out_d = nc.dram_tensor("out", [bs * N, T, D], F32, kind="ExternalOutput")

    cshapes = {
        "wqkT": ([D, H * D], BF), "wvo": ([D, H * D], BF),
        "bma": ([11, 120], BF), "bmb": ([11, 120], BF),
        "w1": ([D, DFF], BF), "w2r": ([128, 4, D], BF),
        "g0": ([D, D], BF), "g12": ([D, 2 * D], BF),
        "a0t": ([N, N], BF), "a1t": ([N, N], BF),
        "gcb4": ([1, 4 * D], BF), "ident": ([128, 128], F32),
        "bvo": ([D, 1], F32), "fb1r": ([128, 4], F32), "fb2": ([D, 1], F32),
        "lng1c": ([D, 1], F32), "lng2c": ([D, 1], F32),
    }
    cd = {k: nc.dram_tensor(k, shp, dt, kind="ExternalInput")
          for k, (shp, dt) in cshapes.items()}

    chunks = []
    off = 0
    while off < TT:
        cw = min(512, TT - off)
        chunks.append((off, cw))
        off += cw

    with tile.TileContext(nc) as tc, ExitStack() as stk:
        nc_ = nc
        singles = stk.enter_context(tc.tile_pool(name="singles", bufs=1))
        bigbf = stk.enter_context(tc.tile_pool(name="bigbf", bufs=1))
        bigf32 = stk.enter_context(tc.tile_pool(name="bigf32", bufs=1))

        # ---- consts to SBUF ----
        csb = {}
        for k, (shp, dt) in cshapes.items():
            if k in ("a0t", "a1t"):
                continue
            t_ = singles.tile(shp, dt, tag=f"c_{k}")
            nc_.sync.dma_start(out=t_, in_=cd[k][...])
            csb[k] = t_
        a_sb = {}
        for k in ("a0t", "a1t"):
            tiles = []
            for mi, (moff, mcnt) in enumerate(NODE_TILES):
                t_ = singles.tile([128, N], BF, tag=f"c_{k}_{mi}")
                nc_.sync.dma_start(out=t_[0:mcnt, :], in_=cd[k][moff:moff + mcnt, :])
                tiles.append(t_)
            a_sb[k] = tiles
        zero_col = singles.tile([128, 1], F32, tag="zero_col")
        nc_.vector.memset(zero_col, 0.0)
        eps_col = singles.tile([128, 1], F32, tag="eps_col")
        nc_.vector.memset(eps_col, LN_EPS)
        zeros512 = singles.tile([128, 512], F32, tag="zeros512")
        nc_.vector.memset(zeros512, 0.0)
        ones_row = singles.tile([1, 128], BF, tag="ones_row")
        nc_.vector.memset(ones_row, 1.0)
        ones128_f = singles.tile([128, 128], F32, tag="ones128_f")
        nc_.vector.memset(ones128_f, 1.0)

        # persistent activations (x1/x2 share one slot; bf tensors share one)
        X1 = bigf32.tile([128, TT], F32, tag="x1")

        cp_i = [0]

        def cp(dst, src):
            """PSUM->SBUF copy alternating DVE/ACT."""
            if cp_i[0] % 2 == 0:
                nc_.vector.tensor_copy(dst, src)
            else:
                nc_.scalar.copy(dst, src)
            cp_i[0] += 1

        # =========== phase 1+2: input, R projection, attention ===========
        with tc.tile_pool(name="att_sb", bufs=2) as att, \
             tc.tile_pool(name="stage_sb", bufs=3) as stage_p, \
             tc.tile_pool(name="xb_p", bufs=1) as xb_p, \
             tc.tile_pool(name="xf_p", bufs=1) as xf_p, \
             tc.tile_pool(name="r_p", bufs=1) as r_p, \
             tc.tile_pool(name="ps_mm1", bufs=1, space="PSUM") as ps_mm1, \
             tc.tile_pool(name="ps_sps", bufs=2, space="PSUM") as ps_sps, \
             tc.tile_pool(name="ps_cps", bufs=2, space="PSUM") as ps_cps:
            x_flat = x_d[...].rearrange("r t d -> (r t) d")  # rows == (b,n,t)
            for b in range(bs):
                # ---- input: stage token-major (b,n,t), transpose to
                # feature-major
                xf = xf_p.tile([128, T * N], F32, tag="xf32")
                xb = xb_p.tile([128, T * N], BF, tag="xbf")
                for o in range(0, T * N, 128):
                    cnt = min(128, T * N - o)
                    st = stage_p.tile([128, 128], F32, tag="stage")
                    nc_.sync.dma_start(
                        out=st[0:cnt, :],
                        in_=x_flat[b * T * N + o:b * T * N + o + cnt, :])
                    tp = ps_mm1.tile([128, 128], F32, tag="mm1")
                    nc_.tensor.transpose(tp[:, 0:cnt], st[0:cnt, :],
                                         csb["ident"][0:cnt, 0:cnt])
                    nc_.vector.tensor_copy(xf[:, o:o + cnt], tp[:, 0:cnt])
                    nc_.scalar.copy(xb[:, o:o + cnt], tp[:, 0:cnt])

                # ---- R projection: R_h^T = wqkT_h-proj of X (feature-major)
                rbuf = r_p.tile([128, H * T * N], BF, tag="rbuf")
                for h in range(H):
                    o = 0
                    while o < T * N:
                        cw = min(512, T * N - o)
                        rp = ps_mm1.tile([128, 512], F32, tag="mm1")
                        nc_.tensor.matmul(rp[:, 0:cw],
                                          lhsT=csb["wqkT"][:, h * D:(h + 1) * D],
                                          rhs=xb[:, o:o + cw],
                                          start=True, stop=True)
                        cp(rbuf[:, h * T * N + o:h * T * N + o + cw], rp[:, 0:cw])
                        o += cw

                # ---- attention over node groups
                for (n0, gn) in GROUPS:
                    gt = gn * 12
                    xb_g = xb[:, n0 * 12:n0 * 12 + gt]
                    xf_g = xf[:, n0 * 12:n0 * 12 + gt]
                    x1_g = X1[:, b * T * N + n0 * 12:b * T * N + n0 * 12 + gt]

                    # Vt (token-major) for all 8 heads: (gt, 1024)
                    vt_ps = ps_mm1.tile([120, 1024], F32, tag="mm1")
                    nc_.tensor.matmul(vt_ps[0:gt, 0:512], lhsT=xb_g,
                                      rhs=csb["wvo"][:, 0:512],
                                      start=True, stop=True)
                    nc_.tensor.matmul(vt_ps[0:gt, 512:1024], lhsT=xb_g,
                                      rhs=csb["wvo"][:, 512:1024],
                                      start=True, stop=True)
                    vt = att.tile([120, 1024], BF, tag="vt")
                    nc_.vector.tensor_copy(vt[0:gt, 0:512], vt_ps[0:gt, 0:512])
                    nc_.scalar.copy(vt[0:gt, 512:1024], vt_ps[0:gt, 512:1024])

                    ctx_ps = ps_cps.tile([128, 120], F32, tag="cps")
                    for half in range(2):
                        sp = ps_sps.tile([120, 480], F32, tag="sps")
                        for hl in range(4):
                            h = half * 4 + hl
                            r_g = rbuf[:, h * T * N + n0 * 12:
                                       h * T * N + n0 * 12 + gt]
                            nc_.tensor.matmul(
                                sp[0:gt, hl * gt:(hl + 1) * gt],
                                lhsT=csb["bma"][:, 0:gt],
                                rhs=csb["bmb"][:, 0:gt],
                                start=True, stop=False)
                            nc_.tensor.matmul(
                                sp[0:gt, hl * gt:(hl + 1) * gt],
                                lhsT=r_g, rhs=xb_g, start=False, stop=True)
                        ph = att.tile([120, 480], BF, tag="ph")
                        nc_.scalar.activation(ph[0:gt, 0:4 * gt],
                                              sp[0:gt, 0:4 * gt], AF.Exp,
                                              bias=zero_col[0:gt], scale=1.0)
                        sums = att.tile([120, 480], F32, tag="sums")
                        nc_.gpsimd.partition_all_reduce(
                            sums[0:gt, 0:4 * gt], ph[0:gt, 0:4 * gt],
                            channels=gt, reduce_op=bass_isa.ReduceOp.add)
                        rec = att.tile([120, 480], F32, tag="sums")
                        nc_.vector.reciprocal_approx_fast(
                            out=rec[0:gt, 0:4 * gt], in_=sums[0:gt, 0:4 * gt])
                        phn = att.tile([120, 480], BF, tag="phn")
                        nc_.vector.tensor_mul(phn[0:gt, 0:4 * gt],
                                              ph[0:gt, 0:4 * gt],
                                              rec[0:gt, 0:4 * gt])
                        for hl in range(4):
                            h = half * 4 + hl
                            nc_.tensor.matmul(
                                ctx_ps[:, 0:gt],
                                lhsT=vt[0:gt, h * D:(h + 1) * D],
                                rhs=phn[0:gt, hl * gt:(hl + 1) * gt],
                                start=(h == 0), stop=(h == H - 1))
                    # residual: X1 = x + attn (+ bvo)
                    nc_.vector.scalar_tensor_tensor(
                        out=x1_g, in0=ctx_ps[:, 0:gt],
                        scalar=csb["bvo"][:, 0:1],
                        in1=xf_g, op0=AL.add, op1=AL.add)

        # =========== LayerNorm helper (feature-major) ===========
        # Column stats broadcast to all partitions by an all-ones (128,128)
        # lhsT matmul; per-element normalize fused into 4 DVE + 3 ACT ops.
        def layer_norm(src, dst_bf, g_col, ident, perm_chunks=None):
            if perm_chunks is None:
                iter_chunks = [(None, o, cw) for (o, cw) in chunks]
            else:
                iter_chunks = perm_chunks
            with tc.tile_pool(name=f"ln_sb_{ident}", bufs=3) as lnp, \
                 tc.tile_pool(name=f"ln_ps_{ident}", bufs=2, space="PSUM") as lps:
                for (pb_, oo, cw) in iter_chunks:
                    o = oo if pb_ is None else pb_ * T * N + oo
                    sq = lnp.tile([128, 512], F32, tag="sqt")
                    nc_.scalar.activation(sq[:, 0:cw], src[:, o:o + cw],
                                          AF.Square, bias=zero_col, scale=1.0)
                    sum_ps = lps.tile([128, 512], F32, tag="lnsum")
                    sq_ps = lps.tile([128, 512], F32, tag="lnsq")
                    nc_.tensor.matmul(sum_ps[:, 0:cw], lhsT=ones128_f,
                                      rhs=src[:, o:o + cw],
                                      start=True, stop=True)
                    nc_.tensor.matmul(sq_ps[:, 0:cw], lhsT=ones128_f,
                                      rhs=sq[:, 0:cw],
                                      start=True, stop=True)
                    t2 = lnp.tile([128, 512], F32, tag="ln_t2")
                    nc_.scalar.activation(t2[:, 0:cw], sum_ps[:, 0:cw],
                                          AF.Square, bias=zero_col,
                                          scale=1.0 / SQRT_D)
                    t3 = lnp.tile([128, 512], F32, tag="ln_t3")
                    nc_.vector.tensor_sub(t3[:, 0:cw], sq_ps[:, 0:cw],
                                          t2[:, 0:cw])
                    sd = lnp.tile([128, 512], F32, tag="ln_sd")
                    nc_.scalar.activation(sd[:, 0:cw], t3[:, 0:cw], AF.Sqrt,
                                          bias=eps_col, scale=1.0 / 128.0)
                    rsig = lnp.tile([128, 512], F32, tag="ln_rs")
                    nc_.vector.reciprocal_approx_fast(out=rsig[:, 0:cw],
                                                      in_=sd[:, 0:cw])
                    t1 = lnp.tile([128, 512], F32, tag="ln_t1")
                    nc_.vector.scalar_tensor_tensor(
                        out=t1[:, 0:cw], in0=sum_ps[:, 0:cw],
                        scalar=-1.0 / 128.0, in1=src[:, o:o + cw],
                        op0=AL.mult, op1=AL.add)
                    if pb_ is None:
                        dst_ap = dst_bf[:, o:o + cw]
                    else:
                        # scatter (n-outer, t-inner) run to (t*N + n) positions
                        v = dst_bf[:, pb_ * T * N:(pb_ + 1) * T * N]
                        v = v.rearrange("d (t n) -> d n t", n=N)
                        n0 = oo // 12
                        dst_ap = v[:, n0:n0 + cw // 12, :]
                    nc_.vector.scalar_tensor_tensor(
                        out=dst_ap, in0=t1[:, 0:cw],
                        scalar=g_col, in1=rsig[:, 0:cw],
                        op0=AL.mult, op1=AL.mult)

        # =========== phase 3: LN1 ===========
        x1nbf = bigbf.tile([128, TT], BF, tag="bigbf")
        layer_norm(X1, x1nbf, csb["lng1c"][:, 0:1], "1")

        # =========== phase 4: FFN + residual ===========
        X2 = bigf32.tile([128, TT], F32, tag="x1")
        with tc.tile_pool(name="ffn_sb", bufs=2) as ffp, \
             tc.tile_pool(name="ffn_ps", bufs=3, space="PSUM") as fps, \
             tc.tile_pool(name="ffn_ps2", bufs=2, space="PSUM") as fps2:
            for (o, cw) in chunks:
                h1 = ffp.tile([128, 4, 512], BF, tag="h1")
                for mt in range(4):
                    fp = fps.tile([128, 512], F32, tag="fps")
                    nc_.tensor.matmul(fp[:, 0:cw],
                                      lhsT=csb["w1"][:, mt * 128:(mt + 1) * 128],
                                      rhs=x1nbf[:, o:o + cw],
                                      start=True, stop=True)
                    nc_.vector.scalar_tensor_tensor(
                        out=h1[:, mt, 0:cw], in0=fp[:, 0:cw],
                        scalar=csb["fb1r"][:, mt:mt + 1],
                        in1=zeros512[:, 0:cw], op0=AL.add, op1=AL.max)
                hp = fps2.tile([128, 512], F32, tag="h2ps")
                for kt in range(4):
                    nc_.tensor.matmul(hp[:, 0:cw],
                                      lhsT=csb["w2r"][:, kt, :],
                                      rhs=h1[:, kt, 0:cw],
                                      start=(kt == 0), stop=(kt == 3))
                nc_.vector.scalar_tensor_tensor(
                    out=X2[:, o:o + cw], in0=hp[:, 0:cw],
                    scalar=csb["fb2"][:, 0:1], in1=x1nbf[:, o:o + cw],
                    op0=AL.add, op1=AL.add)

        # =========== phase 5: LN2 ===========
        # Z is written in (b, t, n) token order (GCN needs node-on-partition
        # tiles); LN2 reads X2 in (b, n, t) order and scatters via strided AP.
        Z = bigbf.tile([128, TT], BF, tag="bigbf")
        ln2_chunks = []
        for b in range(bs):
            o = 0
            while o < N * 12:
                nn = min(42, N - o // 12)
                ln2_chunks.append((b, o, nn * 12))
                o += nn * 12
        layer_norm(X2, Z, csb["lng2c"][:, 0:1], "2", perm_chunks=ln2_chunks)

        # =========== phase 6: GCN ===========
        with tc.tile_pool(name="gcn_sb", bufs=2) as gcp, \
             tc.tile_pool(name="gcn_stg", bufs=3) as gst, \
             tc.tile_pool(name="gcn_pps", bufs=3, space="PSUM") as pps, \
             tc.tile_pool(name="gcn_mps", bufs=3, space="PSUM") as mps:
            for b in range(bs):
                def blk(t, noff, cnt):
                    o = b * T * N + t * N + noff
                    return Z[:, o:o + cnt]

                pb = gcp.tile([128, 3, 2, T, 128], BF, tag="pb")
                for t in range(T):
                    for nt, (noff, cnt) in enumerate(NODE_TILES):
                        pp = pps.tile([128, 256], F32, tag="pps")
                        nc_.tensor.matmul(pp[0:cnt, :], lhsT=blk(t, noff, cnt),
                                          rhs=csb["g12"][:, :],
                                          start=True, stop=True)
                        cp(pb[0:cnt, nt, :, t, :],
                           pp[0:cnt, :].rearrange("p (s e) -> p s e", s=2))
                for ntile, (noff, cnt_n) in enumerate(NODE_TILES):
                    for c in range(3):
                        mx = mps.tile([128, 512], F32, tag="mps")
                        first = True
                        for sup, akey in ((0, "a0t"), (1, "a1t")):
                            for mt, (moff, cnt_m) in enumerate(NODE_TILES):
                                nc_.tensor.matmul(
                                    mx[0:cnt_n, :],
                                    lhsT=a_sb[akey][mt][0:cnt_m,
                                                        noff:noff + cnt_n],
                                    rhs=pb[0:cnt_m, mt, sup,
                                           4 * c:4 * c + 4, :],
                                    start=first, stop=False,
                                    skip_group_check=True)
                                first = False
                        for tj in range(4):
                            t = 4 * c + tj
                            nc_.tensor.matmul(
                                mx[0:cnt_n, tj * 128:(tj + 1) * 128],
                                lhsT=blk(t, noff, cnt_n),
                                rhs=csb["g0"][:, :],
                                start=False, stop=False,
                                skip_group_check=True)
                        nc_.tensor.matmul(
                            mx[0:cnt_n, :],
                            lhsT=ones_row[0:1, 0:cnt_n],
                            rhs=csb["gcb4"][0:1, :],
                            start=False, stop=True,
                            skip_group_check=True)
                        stg = gst.tile([128, 512], F32, tag="ostg")
                        cp(stg[0:cnt_n, :], mx[0:cnt_n, :])
                        nc_.sync.dma_start(
                            out=out_d[b * N + noff:b * N + noff + cnt_n,
                                      4 * c:4 * c + 4, :],
                            in_=stg[0:cnt_n, 0:512]
                            .rearrange("n (t d) -> n t d", d=128))

    nc.compile()
    return nc


_CACHE = {}


def _get_module(bs):
    if bs not in _CACHE:
        _CACHE[bs] = build_module(bs)
    return _CACHE[bs]


def kernel(**inputs):
    from concourse.bass_utils import run_bass_kernel_spmd

    x = np.asarray(inputs["x"], np.float32)
    BN = x.shape[0]
    B = BN // N
    bs = B // NCORES
    consts = make_consts(inputs)
    nc = _get_module(bs)

    in_maps = []
    for c in range(NCORES):
        m = dict(consts)
        m["x"] = _r(x[c * bs * N:(c + 1) * bs * N])
        in_maps.append(m)
    res = run_bass_kernel_spmd(nc, in_maps, list(range(NCORES)))
    out = np.concatenate([res.results[c]["out"] for c in range(NCORES)], axis=0)
    return out.astype(np.float32)


# revision 25
# speedup vs baseline: 2296.6383x; 2296.6383x over previous
"""Trainium2 Bass kernel for nn_GCEncoderLayer_78400333021790.

GC encoder layer: per-node MHA over T=12 steps + FFN (both with residual+LN),
then a 3-support graph convolution over the 325-node sensor graph.

Strategy (data-parallel over batch B=32 -> 4 batches per core, 8 cores):
  - token order per core: (b, t, n); activations kept feature-major
    X^T = (d=128 partitions, tokens free) so every projection is a natural
    PE matmul.
  - MHA algebra folded on CPU:  S^T = (X Wqk^T) X^T with Wqk = Wq Wk^T/sqrt(128)
    (bq=bk=0 per spec), Vt = X (Wv Wo) so the output projection disappears.
  - groups of 10 nodes (120 tokens) per attention step; block-diagonal mask
    realized as a rank-11 matmul pre-loaded into PSUM (exp underflows to 0).
  - softmax normalization: exp (ACT) -> partition_all_reduce (GPSIMD) ->
    reciprocal_approx_fast + multiply (DVE).
  - LayerNorm in feature-major: column sums via ones-matmuls into
    tile_position-rotated PSUM rows, reshaped to (128, x) tiles via SBUF DMA,
    per-token scale/shift broadcast back through K=1 matmuls.
  - GCN: out = Z G0 + A0 (Z G1) + A1 (Z G2) + bias with dense A built on CPU;
    the node-mix contraction runs on token-major tiles (node on partition)
    which the (b, t, n) token order provides for free.
"""

import os
import sys

for _p in ("/opt/trn_rl_repo", "/root/.axon_site/_ro/trn_rl_repo"):
    if os.path.isdir(_p) and _p not in sys.path:
        sys.path.insert(0, _p)

from contextlib import ExitStack

import ml_dtypes
import numpy as np

import concourse.bass as bass
import concourse.bass_isa as bass_isa
import concourse.tile as tile
from concourse import bacc, mybir

N = 325
T = 12
D = 128
H = 8
DFF = 512
NCORES = 8
B_TOT = 32
LN_EPS = 1e-3
SQRT_D = float(np.sqrt(128.0))

BF = mybir.dt.bfloat16
F32 = mybir.dt.float32
F32R = mybir.dt.float32r
AL = mybir.AluOpType
AF = mybir.ActivationFunctionType
bf16 = ml_dtypes.bfloat16

NODE_TILES = [(0, 128), (128, 128), (256, 69)]
GROUPS = [(i * 10, 10) for i in range(32)] + [(320, 5)]
BIG = 173.0  # sqrt(~30000); exp(-BIG^2) == 0 in fp32


def _r(x):
    return np.ascontiguousarray(x)


def _bf(x):
    return _r(np.asarray(x, np.float32).astype(bf16))


def make_consts(inp):
    """CPU-side weight folding. Returns dict of extra dram inputs (shared
    across cores)."""
    Wq = np.asarray(inp["Wq"], np.float32)
    Wk = np.asarray(inp["Wk"], np.float32)
    Wv = np.asarray(inp["Wv"], np.float32)
    Wo = np.asarray(inp["Wo"], np.float32)
    bv = np.asarray(inp["bv"], np.float32)
    bo = np.asarray(inp["bo"], np.float32)

    # wqkT[:, h*D:(h+1)*D][d, e] = Wqk_h[e, d],  Wqk_h = Wq_h Wk_h^T / sqrt(D)
    wqkT = np.empty((D, H * D), np.float32)
    wvo = np.empty((D, H * D), np.float32)
    for h in range(H):
        wqk_h = (Wq[:, h, :] @ Wk[:, h, :].T) / SQRT_D  # (D, D)
        wqkT[:, h * D:(h + 1) * D] = wqk_h.T
        wvo[:, h * D:(h + 1) * D] = Wv[:, h, :] @ Wo[h]  # (D, D)
    bvo = (np.einsum("hk,hkd->d", bv, Wo) + bo).astype(np.float32)

    # block-diag mask via rank-11 outer product: sum_p A[p,s] B[p,t]
    # = BIG^2*[node(s)==node(t)] - BIG^2
    bma = np.zeros((11, 120), np.float32)
    bmb = np.zeros((11, 120), np.float32)
    for blk in range(10):
        bma[blk, blk * 12:(blk + 1) * 12] = BIG
        bmb[blk, blk * 12:(blk + 1) * 12] = BIG
    bma[10, :] = BIG
    bmb[10, :] = -BIG

    A0 = np.zeros((N, N), np.float32)
    A1 = np.zeros((N, N), np.float32)
    np.add.at(A0, (np.asarray(inp["sup0_rows"]), np.asarray(inp["sup0_cols"])),
              np.asarray(inp["sup0_vals"], np.float32))
    np.add.at(A1, (np.asarray(inp["sup1_rows"]), np.asarray(inp["sup1_cols"])),
              np.asarray(inp["sup1_vals"], np.float32))

    G = np.asarray(inp["gc_kernel"], np.float32)  # (3D, D), rows ordered (d, m)
    G0, G1, G2 = G[0::3], G[1::3], G[2::3]  # each (D, D)

    w2 = np.asarray(inp["ffn_W2"], np.float32)  # (DFF, D)
    w2r = w2.reshape(4, 128, D).transpose(1, 0, 2)  # (128, 4, D)
    fb1r = np.asarray(inp["ffn_b1"], np.float32).reshape(4, 128).T  # (128, 4)

    consts = {
        "wqkT": _bf(wqkT),
        "wvo": _bf(wvo),
        "bma": _bf(bma),
        "w1": _bf(inp["ffn_W1"]),
        "w2r": _bf(w2r),

        "a0t": _bf(A0.T),
        "a1t": _bf(A1.T),
        "gcb4": _bf(np.tile(np.asarray(inp["gc_bias"], np.float32), 4)[None, :]),
        "bmb8": _bf(np.concatenate(
            [np.pad(bmb, ((0, 0), (0, 8))) for _ in range(8)], axis=1)),
        "bmb8s": _bf(np.concatenate(
            [np.pad(bmb[:, 0:60], ((0, 0), (0, 68))) for _ in range(8)],
            axis=1)),
        "g012": _bf(np.concatenate([G1, G2, G0], axis=1)),
        "ident": _r(np.eye(128, dtype=np.float32)),
        "bvo": _r(bvo[:, None]),
        "fb1r": _r(fb1r),
        "fb2": _r(np.asarray(inp["ffn_b2"], np.float32)[:, None]),
        "lng1c": _r(np.asarray(inp["ln1_g"], np.float32)[:, None]),
        "lng2c": _r(np.asarray(inp["ln2_g"], np.float32)[:, None]),
    }
    return consts


def build_module(bs):
    """Emit the Bass/Tile program for one core handling `bs` batches."""
    TT = bs * T * N
    nc = bacc.Bacc("TRN2")

    x_d = nc.dram_tensor("x", [bs * N, T, D], F32, kind="ExternalInput")
    out_d = nc.dram_tensor("out", [bs * N, T, D], F32, kind="ExternalOutput")

    cshapes = {
        "wqkT": ([D, H * D], BF), "wvo": ([D, H * D], BF),
        "bma": ([11, 120], BF),
        "w1": ([D, DFF], BF), "w2r": ([128, 4, D], BF),
        
        "a0t": ([N, N], BF), "a1t": ([N, N], BF),
        "gcb4": ([1, 4 * D], BF), "ident": ([128, 128], F32),
        "bmb8": ([11, 8 * 128], BF), "bmb8s": ([11, 8 * 128], BF), "g012": ([D, 3 * D], BF),
        "bvo": ([D, 1], F32), "fb1r": ([128, 4], F32), "fb2": ([D, 1], F32),
        "lng1c": ([D, 1], F32), "lng2c": ([D, 1], F32),
    }
    cd = {k: nc.dram_tensor(k, shp, dt, kind="ExternalInput")
          for k, (shp, dt) in cshapes.items()}

    chunks = []
    off = 0
    while off < TT:
        cw = min(512, TT - off)
        chunks.append((off, cw))
        off += cw

    with tile.TileContext(nc) as tc, ExitStack() as stk:
        nc_ = nc
        singles = stk.enter_context(tc.tile_pool(name="singles", bufs=1))
        bigf32 = stk.enter_context(tc.tile_pool(name="bigf32", bufs=1))

        # ---- consts to SBUF ----
        csb = {}
        for k, (shp, dt) in cshapes.items():
            if k in ("a0t", "a1t"):
                continue
            t_ = singles.tile(shp, dt, tag=f"c_{k}")
            nc_.sync.dma_start(out=t_, in_=cd[k][...])
            csb[k] = t_
        a_sb = {}
        for k in ("a0t", "a1t"):
            tiles = []
            for mi, (moff, mcnt) in enumerate(NODE_TILES):
                t_ = singles.tile([128, N], BF, tag=f"c_{k}_{mi}")
                nc_.sync.dma_start(out=t_[0:mcnt, :], in_=cd[k][moff:moff + mcnt, :])
                tiles.append(t_)
            a_sb[k] = tiles
        zero_col = singles.tile([128, 1], F32, tag="zero_col")
        nc_.vector.memset(zero_col, 0.0)
        eps_col = singles.tile([128, 1], F32, tag="eps_col")
        nc_.vector.memset(eps_col, LN_EPS)
        ones_row = singles.tile([1, 128], BF, tag="ones_row")
        nc_.vector.memset(ones_row, 1.0)
        ones128_f = singles.tile([128, 128], F32, tag="ones128_f")
        nc_.vector.memset(ones128_f, 1.0)
        ones128_b = singles.tile([128, 128], BF, tag="ones128_b")
        nc_.vector.memset(ones128_b, 1.0)

        # persistent activations (x1/x2 share one slot; bf tensors share one)
        X1 = bigf32.tile([128, TT], F32, tag="x1")

        cp_i = [0]

        def cp(dst, src):
            """PSUM->SBUF copy alternating DVE/ACT."""
            if cp_i[0] % 2 == 0:
                nc_.vector.tensor_copy(dst, src)
            else:
                nc_.scalar.copy(dst, src)
            cp_i[0] += 1

        # =========== phase 1+2: input, R projection, attention ===========
        with tc.tile_pool(name="att_sb", bufs=2) as att, \
             tc.tile_pool(name="stage_sb", bufs=3) as stage_p, \
             tc.tile_pool(name="xb_p", bufs=2) as xb_p, \
             tc.tile_pool(name="xf_p", bufs=1) as xf_p, \
             tc.tile_pool(name="r_p", bufs=1) as r_p, \
             tc.tile_pool(name="ps_io", bufs=2, space="PSUM") as ps_io, \
             tc.tile_pool(name="ps_vps", bufs=1, space="PSUM") as ps_vps, \
             tc.tile_pool(name="ps_sps", bufs=1, space="PSUM") as ps_sps, \
             tc.tile_pool(name="ps_cps", bufs=2, space="PSUM") as ps_cps:
            x_flat = x_d[...].rearrange("r t d -> (r t) d")  # rows == (b,n,t)
            for b in range(bs):
                # ---- input: stage token-major (b,n,t), transpose to
                # feature-major
                xf = xf_p.tile([128, T * N], F32, tag="xf32")
                xb = xb_p.tile([128, T * N], BF, tag="xbf")
                for o in range(0, T * N, 128):
                    cnt = min(128, T * N - o)
                    st = stage_p.tile([128, 128], F32, tag="stage")
                    nc_.sync.dma_start(
                        out=st[0:cnt, :],
                        in_=x_flat[b * T * N + o:b * T * N + o + cnt, :])
                    tp = ps_io.tile([128, 512], F32, tag="io")
                    nc_.tensor.transpose(tp[:, 0:cnt], st[0:cnt, :],
                                         csb["ident"][0:cnt, 0:cnt])
                    nc_.vector.tensor_copy(xf[:, o:o + cnt], tp[:, 0:cnt])
                    nc_.scalar.copy(xb[:, o:o + cnt], tp[:, 0:cnt])

                # ---- R projection: R_h^T = wqkT_h-proj of X (feature-major)
                rbuf = r_p.tile([128, H * T * N], BF, tag="rbuf")
                for h in range(H):
                    o = 0
                    while o < T * N:
                        cw = min(512, T * N - o)
                        rp = ps_io.tile([128, 512], F32, tag="io")
                        nc_.tensor.matmul(rp[:, 0:cw],
                                          lhsT=csb["wqkT"][:, h * D:(h + 1) * D],
                                          rhs=xb[:, o:o + cw],
                                          start=True, stop=True)
                        nc_.scalar.copy(
                            rbuf[:, h * T * N + o:h * T * N + o + cw],
                            rp[:, 0:cw])
                        o += cw

                # ---- attention over node groups
                for (n0, gn) in GROUPS:
                    gt = gn * 12
                    xb_g = xb[:, n0 * 12:n0 * 12 + gt]
                    xf_g = xf[:, n0 * 12:n0 * 12 + gt]
                    x1_g = X1[:, b * T * N + n0 * 12:b * T * N + n0 * 12 + gt]

                    # Vt (token-major) for all 8 heads: (gt, 1024)
                    vt_ps = ps_vps.tile([120, 1024], F32, tag="vps")
                    nc_.tensor.matmul(vt_ps[0:gt, 0:512], lhsT=xb_g,
                                      rhs=csb["wvo"][:, 0:512],
                                      start=True, stop=True)
                    nc_.tensor.matmul(vt_ps[0:gt, 512:1024], lhsT=xb_g,
                                      rhs=csb["wvo"][:, 512:1024],
                                      start=True, stop=True)
                    vt = att.tile([120, 1024], BF, tag="vt")
                    nc_.scalar.copy(vt[0:gt, 0:512], vt_ps[0:gt, 0:512])
                    nc_.scalar.copy(vt[0:gt, 512:1024], vt_ps[0:gt, 512:1024])

                    ctx_ps = ps_cps.tile([128, 120], F32, tag="cps")
                    sp = ps_sps.tile([120, 1024], F32, tag="sps")
                    bm8 = csb["bmb8"] if gn == 10 else csb["bmb8s"]
                    nc_.tensor.matmul(sp[0:gt, 0:512],
                                      lhsT=csb["bma"][:, 0:gt],
                                      rhs=bm8[:, 0:512],
                                      start=True, stop=False,
                                      skip_group_check=True)
                    nc_.tensor.matmul(sp[0:gt, 512:1024],
                                      lhsT=csb["bma"][:, 0:gt],
                                      rhs=bm8[:, 512:1024],
                                      start=True, stop=False,
                                      skip_group_check=True)
                    for h in range(H):
                        r_g = rbuf[:, h * T * N + n0 * 12:
                                   h * T * N + n0 * 12 + gt]
                        nc_.tensor.matmul(
                            sp[0:gt, h * 128:h * 128 + gt],
                            lhsT=r_g, rhs=xb_g, start=False, stop=True,
                            skip_group_check=True)
                    ph = att.tile([120, 1024], BF, tag="ph")
                    nc_.scalar.activation(ph[0:gt, 0:1024],
                                          sp[0:gt, 0:1024], AF.Exp,
                                          bias=zero_col[0:gt], scale=1.0)
                    sums = att.tile([120, 1024], F32, tag="sums")
                    nc_.gpsimd.partition_all_reduce(
                        sums[0:gt, 0:1024], ph[0:gt, 0:1024],
                        channels=gt, reduce_op=bass_isa.ReduceOp.add)
                    rec = att.tile([120, 1024], F32, tag="rec")
                    nc_.vector.reciprocal_approx_fast(
                        out=rec[0:gt, 0:1024], in_=sums[0:gt, 0:1024])
                    phn = att.tile([120, 1024], BF, tag="phn")
                    nc_.vector.tensor_mul(phn[0:gt, 0:1024],
                                          ph[0:gt, 0:1024],
                                          rec[0:gt, 0:1024])
                    for h in range(H):
                        nc_.tensor.matmul(
                            ctx_ps[:, 0:gt],
                            lhsT=vt[0:gt, h * D:(h + 1) * D],
                            rhs=phn[0:gt, h * 128:h * 128 + gt],
                            start=(h == 0), stop=(h == H - 1))
                    # residual: X1 = x + attn (+ bvo)
                    nc_.vector.scalar_tensor_tensor(
                        out=x1_g, in0=ctx_ps[:, 0:gt],
                        scalar=csb["bvo"][:, 0:1],
                        in1=xf_g, op0=AL.add, op1=AL.add)

        # =========== LayerNorm helper (feature-major) ===========
        # Column stats broadcast to all partitions by an all-ones (128,128)
        # lhsT matmul; per-element normalize fused into 4 DVE + 3 ACT ops.
        def layer_norm(src, dst_bf, g_col, ident, perm_chunks=None):
            if perm_chunks is None:
                iter_chunks = [(None, o, cw) for (o, cw) in chunks]
            else:
                iter_chunks = perm_chunks
            with tc.tile_pool(name=f"ln_sb_{ident}", bufs=3) as lnp, \
                 tc.tile_pool(name=f"ln_ps_{ident}", bufs=3, space="PSUM") as lps:
                for (pb_, oo, cw) in iter_chunks:
                    o = oo if pb_ is None else pb_ * T * N + oo
                    sq = lnp.tile([128, 512], BF, tag="sqt")
                    nc_.scalar.activation(sq[:, 0:cw], src[:, o:o + cw],
                                          AF.Square, bias=zero_col, scale=1.0)
                    sum_ps = lps.tile([128, 512], F32, tag="lnsum")
                    sq_ps = lps.tile([128, 512], F32, tag="lnsq")
                    nc_.tensor.matmul(sum_ps[:, 0:cw], lhsT=ones128_f,
                                      rhs=src[:, o:o + cw],
                                      start=True, stop=True)
                    nc_.tensor.matmul(sq_ps[:, 0:cw], lhsT=ones128_b,
                                      rhs=sq[:, 0:cw],
                                      start=True, stop=True)
                    t2 = lnp.tile([128, 512], F32, tag="ln_t2")
                    nc_.scalar.activation(t2[:, 0:cw], sum_ps[:, 0:cw],
                                          AF.Square, bias=zero_col,
                                          scale=1.0 / SQRT_D)
                    t3 = lnp.tile([128, 512], F32, tag="ln_t3")
                    nc_.vector.tensor_sub(t3[:, 0:cw], sq_ps[:, 0:cw],
                                          t2[:, 0:cw])
                    sd = lnp.tile([128, 512], F32, tag="ln_sd")
                    nc_.scalar.activation(sd[:, 0:cw], t3[:, 0:cw], AF.Sqrt,
                                          bias=eps_col, scale=1.0 / 128.0)
                    rsig = lnp.tile([128, 512], F32, tag="ln_rs")
                    nc_.vector.reciprocal_approx_fast(out=rsig[:, 0:cw],
                                                      in_=sd[:, 0:cw])
                    t1 = lnp.tile([128, 512], F32, tag="ln_t1")
                    nc_.vector.scalar_tensor_tensor(
                        out=t1[:, 0:cw], in0=sum_ps[:, 0:cw],
                        scalar=-1.0 / 128.0, in1=src[:, o:o + cw],
                        op0=AL.mult, op1=AL.add)
                    if pb_ is None:
                        dst_ap = dst_bf[:, o:o + cw]
                    else:
                        # scatter (n-outer, t-inner) run to (t*N + n) positions
                        v = dst_bf[:, pb_ * T * N:(pb_ + 1) * T * N]
                        v = v.rearrange("d (t n) -> d n t", n=N)
                        n0 = oo // 12
                        dst_ap = v[:, n0:n0 + cw // 12, :]
                    nc_.vector.scalar_tensor_tensor(
                        out=dst_ap, in0=t1[:, 0:cw],
                        scalar=g_col, in1=rsig[:, 0:cw],
                        op0=AL.mult, op1=AL.mult)

        # =========== phase 3: LN1 ===========
        bigbf = stk.enter_context(tc.tile_pool(name="bigbf", bufs=1))
        x1nbf = bigbf.tile([128, TT], BF, tag="bigbf")
        layer_norm(X1, x1nbf, csb["lng1c"][:, 0:1], "1")

        # =========== phase 4: FFN + residual ===========
        X2 = bigf32.tile([128, TT], F32, tag="x1")
        with tc.tile_pool(name="ffn_sb", bufs=2) as ffp, \
             tc.tile_pool(name="ffn_ps", bufs=4, space="PSUM") as fps, \
             tc.tile_pool(name="ffn_ps2", bufs=3, space="PSUM") as fps2:
            for (o, cw) in chunks:
                h1 = ffp.tile([128, 4, 512], BF, tag="h1")
                for mt in range(4):
                    fp = fps.tile([128, 512], F32, tag="fps")
                    nc_.tensor.matmul(fp[:, 0:cw],
                                      lhsT=csb["w1"][:, mt * 128:(mt + 1) * 128],
                                      rhs=x1nbf[:, o:o + cw],
                                      start=True, stop=True)
                    nc_.vector.tensor_scalar(
                        out=h1[:, mt, 0:cw], in0=fp[:, 0:cw],
                        scalar1=csb["fb1r"][:, mt:mt + 1], scalar2=0.0,
                        op0=AL.add, op1=AL.max)
                hp = fps2.tile([128, 512], F32, tag="h2ps")
                for kt in range(4):
                    nc_.tensor.matmul(hp[:, 0:cw],
                                      lhsT=csb["w2r"][:, kt, :],
                                      rhs=h1[:, kt, 0:cw],
                                      start=(kt == 0), stop=(kt == 3))
                nc_.vector.scalar_tensor_tensor(
                    out=X2[:, o:o + cw], in0=hp[:, 0:cw],
                    scalar=csb["fb2"][:, 0:1], in1=x1nbf[:, o:o + cw],
                    op0=AL.add, op1=AL.add)

        # =========== phase 5: LN2 ===========
        # Z is written in (b, t, n) token order (GCN needs node-on-partition
        # tiles); LN2 reads X2 in (b, n, t) order and scatters via strided AP.
        Z = bigbf.tile([128, TT], BF, tag="bigbf")
        ln2_chunks = []
        for b in range(bs):
            o = 0
            while o < N * 12:
                nn = min(42, N - o // 12)
                ln2_chunks.append((b, o, nn * 12))
                o += nn * 12
        layer_norm(X2, Z, csb["lng2c"][:, 0:1], "2", perm_chunks=ln2_chunks)

        # =========== phase 6: GCN ===========
        with tc.tile_pool(name="gcn_sb", bufs=2) as gcp, \
             tc.tile_pool(name="gcn_stg", bufs=3) as gst, \
             tc.tile_pool(name="gcn_pps", bufs=4, space="PSUM") as pps, \
             tc.tile_pool(name="gcn_mps", bufs=4, space="PSUM") as mps:
            for b in range(bs):
                def blk(t, noff, cnt):
                    o = b * T * N + t * N + noff
                    return Z[:, o:o + cnt]

                pb = gcp.tile([128, 3, 3, T, 128], BF, tag="pb")
                for t in range(T):
                    for nt, (noff, cnt) in enumerate(NODE_TILES):
                        pp = pps.tile([128, 384], F32, tag="pps")
                        nc_.tensor.matmul(pp[0:cnt, :], lhsT=blk(t, noff, cnt),
                                          rhs=csb["g012"][:, :],
                                          start=True, stop=True)
                        cp(pb[0:cnt, nt, :, t, :],
                           pp[0:cnt, :].rearrange("p (s e) -> p s e", s=3))
                for ntile, (noff, cnt_n) in enumerate(NODE_TILES):
                    for c in range(3):
                        mx = mps.tile([128, 512], F32, tag="mps")
                        first = True
                        for sup, akey in ((0, "a0t"), (1, "a1t")):
                            for mt, (moff, cnt_m) in enumerate(NODE_TILES):
                                nc_.tensor.matmul(
                                    mx[0:cnt_n, :],
                                    lhsT=a_sb[akey][mt][0:cnt_m,
                                                        noff:noff + cnt_n],
                                    rhs=pb[0:cnt_m, mt, sup,
                                           4 * c:4 * c + 4, :],
                                    start=first, stop=False,
                                    skip_group_check=True)
                                first = False
                        nc_.tensor.matmul(
                            mx[0:cnt_n, :],
                            lhsT=ones_row[0:1, 0:cnt_n],
                            rhs=csb["gcb4"][0:1, :],
                            start=False, stop=True,
                            skip_group_check=True)
                        stg = gst.tile([128, 512], F32, tag="ostg")
                        nc_.vector.scalar_tensor_tensor(
                            out=stg[0:cnt_n, :], in0=mx[0:cnt_n, :],
                            scalar=1.0,
                            in1=pb[0:cnt_n, ntile, 2, 4 * c:4 * c + 4, :],
                            op0=AL.mult, op1=AL.add)
                        nc_.sync.dma_start(
                            out=out_d[b * N + noff:b * N + noff + cnt_n,
                                      4 * c:4 * c + 4, :],
                            in_=stg[0:cnt_n, 0:512]
                            .rearrange("n (t d) -> n t d", d=128))

    nc.compile()
    return nc


_CACHE = {}


def _get_module(bs):
    if bs not in _CACHE:
        _CACHE[bs] = build_module(bs)
    return _CACHE[bs]


def kernel(**inputs):
    from concourse.bass_utils import run_bass_kernel_spmd

    x = np.asarray(inputs["x"], np.float32)
    BN = x.shape[0]
    B = BN // N
    bs = B // NCORES
    consts = make_consts(inputs)
    nc = _get_module(bs)

    in_maps = []
    for c in range(NCORES):
        m = dict(consts)
        m["x"] = _r(x[c * bs * N:(c + 1) * bs * N])
        in_maps.append(m)
    res = run_bass_kernel_spmd(nc, in_maps, list(range(NCORES)))
    out = np.concatenate([res.results[c]["out"] for c in range(NCORES)], axis=0)
    return out.astype(np.float32)
